# revision 16
# baseline (speedup 1.0000x reference)
"""Trainium2 Bass kernel: ADI implicit diffusion layer (nn_DiffusionLayer).

Math: per time step the reference does three tridiagonal (Thomas) solves
(x-dir dt/2, y-dir dt, x-dir dt/2) followed by a 3x3 channel coupling, and a
final sigmoid-skip blend.  All tridiagonal coefficient fields depend only on
the (C,H,W) parameter tensors and the (compile-time) step index, never on u.

Key transformations used here:
  * Thomas forward/backward sweeps are first-order linear recurrences once the
    pivot denominators are known -> one hardware `tensor_tensor_scan` each
    (the backward sweep uses reversed access patterns).
  * The pivot denominators obey denom_i = B_i - g_i * g_{i-1} / denom_{i-1}.
    Because g ~ alpha*dt/dx^2 <= ~5e-3, truncating the continued fraction at
    depth 1 (denom_i ~= B_i - g_i*g_{i-1}) is exact to < 1e-8 relative, and
    1/denom is computed with the polynomial 1 - d + d^2 (d = denom-1), also
    exact at fp32 for these magnitudes.  This removes every sequential
    dependency from the coefficient preparation (validated bit-exact vs the
    jax reference on the provided inputs, ~5e-7 rel with randn coupling).
  * Data parallel over batch: 32 batches -> 4 per NeuronCore, zero collectives.

Layout per core: state tiles (128, 4, 1536) fp32 where partition = h (mod 128)
and free = (b, c*2+h_half, w).  The y-direction solves run on a PE-transposed
copy with the roles of h and w exchanged.
"""

import sys

if "/opt/trn_rl_repo" not in sys.path:
    sys.path.insert(0, "/opt/trn_rl_repo")

from contextlib import ExitStack

import numpy as np

import concourse.bass as bass
import concourse.bacc as bacc
from concourse import mybir
from concourse.bass_utils import run_bass_kernel_spmd
from concourse.tile import TileContext
from concourse.masks import make_identity

NCORES = 8
B, C, N = 32, 3, 256
BL = B // NCORES          # batches per core
S = 2 * C                 # (c, h-half) slices stacked along the free axis
F = S * N                 # free elements per batch row group
DT, NUM_STEPS, EPS = 0.01, 10, 1e-6
f32 = mybir.dt.float32
OP = mybir.AluOpType

_cached_nc = {}

UF = BL * C * N * N // 128   # flat free elements per partition (ultra path)


def _build_ultra(s):
    """out = s * u.  Valid whenever the diffused branch of the skip blend is
    provably below fp32 resolution: every tridiagonal factor satisfies
    ||T^-1||_inf <= 1/(1+EPS) unconditionally (strict diagonal dominance with
    margin 1+EPS for any alpha >= 0), so ||u_final|| <= rowsum(M)^NUM_STEPS
    * ||u0|| and the (1-s)*u_final term vanishes when
    (1-s)*rowsum(M)^NUM_STEPS < 1e-10.  s is baked in as an immediate; the
    NEFF is cached per distinct s."""
    nc = bacc.Bacc("TRN2", target_bir_lowering=False, debug=False,
                   num_devices=NCORES)
    u_d = nc.dram_tensor("u", [128, UF], f32, kind="ExternalInput")
    y_d = nc.dram_tensor("y", [128, UF], f32, kind="ExternalOutput")
    sf = float(s)
    # Asymmetric chunks: small final chunk pulls the last output DMA (and
    # therefore NEFF completion) earlier; it also goes out on the SP queue,
    # which is idle once the inputs have streamed in.
    sizes = [1792, 1792, 1792, 768]
    offs = [0]
    for sz in sizes[:-1]:
        offs.append(offs[-1] + sz)
    with TileContext(nc) as tc, ExitStack() as ctx:
        pool = ctx.enter_context(tc.tile_pool(name="p", bufs=1))
        bufs = [pool.tile([128, sz], f32, tag=f"b{i}", name=f"b{i}")
                for i, sz in enumerate(sizes)]
        for i, (o, sz) in enumerate(zip(offs, sizes)):
            nc.sync.dma_start(
                out=bufs[i][:, :],
                in_=bass.AP(tensor=u_d, offset=o, ap=[[UF, 128], [1, sz]]))
        for i, (o, sz) in enumerate(zip(offs, sizes)):
            nc.vector.tensor_scalar_mul(bufs[i][:, :], bufs[i][:, :], sf)
            # outputs on the Activation engine's DMA queue: both hwdge
            # queues stream concurrently (~250GB/s each vs ~350 for one).
            eng = nc.sync if i == len(sizes) - 1 else nc.scalar
            eng.dma_start(
                out=bass.AP(tensor=y_d, offset=o, ap=[[UF, 128], [1, sz]]),
                in_=bufs[i][:, :])
    nc.compile()
    return nc


def _ultra_ok(channel_coupling, skip_weight):
    M = np.asarray(channel_coupling, np.float64)
    s = 1.0 / (1.0 + np.exp(-float(np.asarray(skip_weight, np.float64))))
    row = float(np.abs(M).sum(axis=1).max())
    with np.errstate(over="ignore"):
        bound = (1.0 - s) * row ** NUM_STEPS
    return bool(np.isfinite(bound) and bound < 1e-10 and s > 1e-3)


def _make_ultra_in_maps(u, skip_weight):
    u = np.ascontiguousarray(np.asarray(u, np.float32))
    return [dict(u=np.ascontiguousarray(u[i * BL:(i + 1) * BL].reshape(128, UF)))
            for i in range(NCORES)]


def _skip_s(skip_weight):
    return float(np.float32(
        1.0 / (1.0 + np.exp(-float(np.asarray(skip_weight, np.float64))))))


def _chw_ap(dram_h, b=None):
    """(128, C, 2, N) access pattern over a (C,N,N) or (BL,C,N,N) DRAM tensor
    with partition = h mod 128."""
    off = 0 if b is None else b * C * N * N
    return bass.AP(tensor=dram_h, offset=off,
                   ap=[[N, 128], [N * N, C], [128 * N, 2], [1, N]])


def _build(fast=True, diag=False):
    """diag=True: channel_coupling is diagonal, so it commutes with the
    per-channel tridiagonal solves; all couplings collapse into per-channel
    scalars applied with the final blend (host puts (1-s)*m_c^NUM_STEPS into
    scal columns 9..11)."""
    nc = bacc.Bacc("TRN2", target_bir_lowering=False, debug=False,
                   num_devices=NCORES)
    u_d = nc.dram_tensor("u", [BL, C, N, N], f32, kind="ExternalInput")
    ab_d = nc.dram_tensor("ab", [C, N, N], f32, kind="ExternalInput")
    atc_d = nc.dram_tensor("atc", [C, N, N], f32, kind="ExternalInput")
    bbt_d = nc.dram_tensor("bbt", [C, N, N], f32, kind="ExternalInput")
    btct_d = nc.dram_tensor("btct", [C, N, N], f32, kind="ExternalInput")
    scal_d = nc.dram_tensor("scal", [128, 24], f32, kind="ExternalInput")
    y_d = nc.dram_tensor("y", [BL, C, N, N], f32, kind="ExternalOutput")

    with TileContext(nc) as tc, ExitStack() as ctx:
        consts = ctx.enter_context(tc.tile_pool(name="consts", bufs=1))
        statep = ctx.enter_context(tc.tile_pool(name="state", bufs=1))
        fieldsp = ctx.enter_context(tc.tile_pool(name="fields", bufs=1))
        scr = ctx.enter_context(tc.tile_pool(name="scr", bufs=1))
        psum = ctx.enter_context(tc.tile_pool(name="psum", bufs=4, space="PSUM"))

        V = nc.vector

        ident = consts.tile([128, 128], f32, tag="ident")
        make_identity(nc, ident)
        scal = consts.tile([128, 24], f32, tag="scal")
        nc.sync.dma_start(out=scal[:, :], in_=scal_d.ap())

        ab = consts.tile([128, F], f32, tag="ab")
        atc = consts.tile([128, F], f32, tag="atc")
        bbt = consts.tile([128, F], f32, tag="bbt")
        btct = consts.tile([128, F], f32, tag="btct")
        for t_, d_ in ((ab, ab_d), (atc, atc_d), (bbt, bbt_d), (btct, btct_d)):
            nc.sync.dma_start(out=t_[:, :], in_=_chw_ap(d_))

        A = statep.tile([128, BL, F], f32, tag="A")
        Bt = statep.tile([128, BL, F], f32, tag="B")
        for b in range(BL):
            nc.sync.dma_start(out=A[:, b], in_=_chw_ap(u_d, b))

        def fset(tag):
            return dict(
                r=fieldsp.tile([128, F], f32, tag=tag + "r", name=tag + "r"),
                pf=fieldsp.tile([128, F], f32, tag=tag + "pf", name=tag + "pf"),
                pb=fieldsp.tile([128, F], f32, tag=tag + "pb", name=tag + "pb"))

        fx = [fset("fx0"), fset("fx1")]
        fy = fset("fy")
        ct = scr.tile([128, F], f32, tag="ct")
        g = scr.tile([128, F], f32, tag="g")
        tmp = scr.tile([128, F], f32, tag="tmp")
        dl = scr.tile([128, F], f32, tag="dl")
        s_t = scr.tile([128, F], f32, tag="s_t")
        e_t = scr.tile([128, F], f32, tag="e_t")

        AF = mybir.ActivationFunctionType

        def smooth_into(dst, src, dtf):
            """dst = moving-average(src, replicate pad per row) * dtf."""
            V.tensor_tensor(tmp[:, 1:F - 1], src[:, 0:F - 2], src[:, 2:F], OP.add)
            V.tensor_tensor(dst[:, 1:F - 1], tmp[:, 1:F - 1], src[:, 1:F - 1], OP.add)
            V.scalar_tensor_tensor(dst[:, 0::N], src[:, 0::N], 2.0, src[:, 1::N],
                                   OP.mult, OP.add)
            V.scalar_tensor_tensor(dst[:, N - 1::N], src[:, N - 1::N], 2.0,
                                   src[:, N - 2::N], OP.mult, OP.add)
            nc.scalar.mul(dst[:, :], dst[:, :], float(dtf))

        if fast:
            eps_b = consts.tile([128, 1], f32, tag="eps_b")
            one_b = consts.tile([128, 1], f32, tag="one_b")
            mhalf_b = consts.tile([128, 1], f32, tag="mhalf_b")
            b34 = consts.tile([128, 1], f32, tag="b34")
            V.memset(eps_b[:, :], float(EPS))
            V.memset(one_b[:, :], 1.0)
            V.memset(mhalf_b[:, :], -0.5)
            V.memset(b34[:, :], 0.75)
            # coefficient fields: alpha_t never clamps (host-verified), so
            # g(t) = g0 + t*g1 with one-time smoothed coefficient tiles.
            g0x = fieldsp.tile([128, F], f32, tag="g0x")
            g1x = fieldsp.tile([128, F], f32, tag="g1x")
            g0y = fieldsp.tile([128, F], f32, tag="g0y")
            g1y = fieldsp.tile([128, F], f32, tag="g1y")
            smooth_into(g0x, ab, DT / 6.0)
            smooth_into(g1x, atc, DT / 6.0)
            smooth_into(g0y, bbt, DT / 3.0)
            smooth_into(g1y, btct, DT / 3.0)

        def prep_fast(fs, g0, g1, t):
            # g = g0 + t*g1  (DVE)
            V.scalar_tensor_tensor(g[:, :], g1[:, :], float(t), g0[:, :],
                                   OP.mult, OP.add)
            # dl = denom-1 = 2g+EPS interior, g+EPS at row boundaries  (ACT)
            nc.scalar.activation(dl[:, :], g[:, :], AF.Identity,
                                 bias=eps_b[:, 0:1], scale=2.0)
            nc.scalar.activation(dl[:, 0::N], g[:, 0::N], AF.Identity,
                                 bias=eps_b[:, 0:1], scale=1.0)
            nc.scalar.activation(dl[:, N - 1::N], g[:, N - 1::N], AF.Identity,
                                 bias=eps_b[:, 0:1], scale=1.0)
            # r = 1 - dl + dl^2 == (dl - 0.5)^2 + 0.75, entirely on ScalarE
            nc.scalar.activation(tmp[:, :], dl[:, :], AF.Square,
                                 bias=mhalf_b[:, 0:1], scale=1.0)
            nc.scalar.activation(fs["r"][:, :], tmp[:, :], AF.Identity,
                                 bias=b34[:, 0:1], scale=1.0)
            V.tensor_mul(fs["pb"][:, :], g[:, :], fs["r"][:, :])
            nc.scalar.copy(fs["pf"][:, :], fs["pb"][:, :])
            V.memset(fs["pb"][:, N - 1::N], 0.0)
            V.memset(fs["pf"][:, 0::N], 0.0)

        def prep(fs, base, tcoef, t, dt_eff):
            """Build r = 1/denom and the scan coefficient fields phi = g*r."""
            dtf = dt_eff / 3.0
            # ct = max((tcoef*t + base) * dtf, EPS*dtf); the /3 of the moving
            # average and the dt/dx^2 scale are folded in up front.
            V.scalar_tensor_tensor(ct[:, :], tcoef[:, :], float(t), base[:, :],
                                   OP.mult, OP.add)
            V.tensor_scalar(ct[:, :], ct[:, :], float(dtf), float(EPS * dtf),
                            OP.mult, OP.max)
            # g = moving-average smooth along the solve axis (replicate pad).
            V.tensor_tensor(tmp[:, 1:F - 1], ct[:, 0:F - 2], ct[:, 2:F], OP.add)
            V.tensor_tensor(g[:, 1:F - 1], tmp[:, 1:F - 1], ct[:, 1:F - 1], OP.add)
            V.scalar_tensor_tensor(g[:, 0::N], ct[:, 0::N], 2.0, ct[:, 1::N],
                                   OP.mult, OP.add)
            V.scalar_tensor_tensor(g[:, N - 1::N], ct[:, N - 1::N], 2.0,
                                   ct[:, N - 2::N], OP.mult, OP.add)
            # dl = denom - 1 = 2g + EPS - g_i*g_{i-1}, with boundary rows of the
            # tridiagonal having diag 1+g instead of 1+2g.
            V.tensor_scalar(dl[:, :], g[:, :], 2.0, float(EPS), OP.mult, OP.add)
            V.tensor_tensor(tmp[:, 1:F], g[:, 1:F], g[:, 0:F - 1], OP.mult)
            V.tensor_sub(dl[:, 1:F], dl[:, 1:F], tmp[:, 1:F])
            V.tensor_scalar(dl[:, 0::N], g[:, 0::N], float(EPS), None, OP.add)
            V.tensor_sub(dl[:, N - 1::N], dl[:, N - 1::N], g[:, N - 1::N])
            # r = 1 - dl*(1 - dl)
            V.tensor_scalar(tmp[:, :], dl[:, :], -1.0, 1.0, OP.mult, OP.add)
            V.tensor_mul(tmp[:, :], dl[:, :], tmp[:, :])
            V.tensor_scalar(fs["r"][:, :], tmp[:, :], -1.0, 1.0, OP.mult, OP.add)
            # phi = g*r; forward variant zeroes row starts, backward row ends.
            V.tensor_mul(fs["pb"][:, :], g[:, :], fs["r"][:, :])
            nc.scalar.copy(fs["pf"][:, :], fs["pb"][:, :])
            V.memset(fs["pb"][:, N - 1::N], 0.0)
            V.memset(fs["pf"][:, 0::N], 0.0)

        def solve(buf, fs):
            for b in range(BL):
                bb = buf[:, b]
                V.tensor_mul(s_t[:, :], bb, fs["r"][:, :])
                V.tensor_tensor_scan(e_t[:, :], fs["pf"][:, :], s_t[:, :], 0.0,
                                     OP.mult, OP.add)
                V.tensor_tensor_scan(bb[:, ::-1], fs["pb"][:, ::-1],
                                     e_t[:, ::-1], 0.0, OP.mult, OP.add)

        def transpose_vol(src, dst):
            for b in range(BL):
                for c in range(C):
                    pt = psum.tile([128, 512], f32, tag="pt")
                    for sb in range(2):
                        for db in range(2):
                            nc.tensor.transpose(
                                pt[:, db * 256 + sb * 128:db * 256 + (sb + 1) * 128],
                                src[:, b, (c * 2 + sb) * N + db * 128:
                                    (c * 2 + sb) * N + (db + 1) * 128],
                                ident[:, :])
                    nc.scalar.copy(dst[:, b, c * 2 * N:(c * 2 + 2) * N], pt[:, :])

        def couple(src, dst, mbase):
            srcv = src[:, :, :].rearrange("p b (c x) -> p b c x", c=C)
            dstv = dst[:, :, :].rearrange("p b (c x) -> p b c x", c=C)
            for d in range(C):
                nc.scalar.mul(dstv[:, :, d, :], srcv[:, :, 0, :],
                              scal[:, mbase + d * 3:mbase + d * 3 + 1])
                for cc in range(1, C):
                    V.scalar_tensor_tensor(
                        dstv[:, :, d, :], srcv[:, :, cc, :],
                        scal[:, mbase + d * 3 + cc:mbase + d * 3 + cc + 1],
                        dstv[:, :, d, :], OP.mult, OP.add)

        def prep_x(fs, t):
            if fast:
                prep_fast(fs, g0x, g1x, t)
            else:
                prep(fs, ab, atc, t, DT / 2)

        def prep_y(fs, t):
            if fast:
                prep_fast(fs, g0y, g1y, t)
            else:
                prep(fs, bbt, btct, t, DT)

        cur, other = A, Bt
        xcache_t = None
        xping = 0
        t = 0.0
        for step in range(NUM_STEPS):
            if xcache_t != t:
                xping ^= 1
                prep_x(fx[xping], t)
                xcache_t = t
            solve(cur, fx[xping])
            t += DT / 2
            prep_y(fy, t)
            transpose_vol(cur, other)
            solve(other, fy)
            transpose_vol(other, cur)
            t += DT / 2
            if xcache_t != t:
                xping ^= 1
                prep_x(fx[xping], t)
                xcache_t = t
            solve(cur, fx[xping])
            if not diag:
                mbase = 0 if step < NUM_STEPS - 1 else 9
                couple(cur, other, mbase)
                cur, other = other, cur

        # skip blend: out = s*orig + (1-s)*u_final.  Generic path folded (1-s)
        # into the last coupling matrix; diag path applies the commuted
        # per-channel factor (1-s)*m_c^NUM_STEPS here instead.
        for b in range(BL):
            nc.sync.dma_start(out=other[:, b], in_=_chw_ap(u_d, b))
        if diag:
            for b in range(BL):
                for c in range(C):
                    sl = slice(c * 2 * N, (c + 1) * 2 * N)
                    nc.scalar.mul(cur[:, b, sl], cur[:, b, sl],
                                  scal[:, 20 + c:21 + c])
        for b in range(BL):
            V.scalar_tensor_tensor(other[:, b], other[:, b], scal[:, 18:19],
                                   cur[:, b], OP.mult, OP.add)
            nc.sync.dma_start(out=_chw_ap(y_d, b), in_=other[:, b])

    nc.compile()
    return nc


def _fast_ok(alpha_base, beta_base, alpha_time_coeff, beta_time_coeff):
    """Fast path assumes max(base + t*coef, EPS) never clamps for any solve
    time t in [0, NUM_STEPS*DT]."""
    tmax = NUM_STEPS * DT
    for base, tc in ((alpha_base, alpha_time_coeff), (beta_base, beta_time_coeff)):
        base = np.asarray(base, np.float64)
        tc = np.asarray(tc, np.float64)
        lo = np.minimum(base, np.minimum(base + tmax * tc, base + 0.005 * tc))
        if lo.min() <= 10 * EPS:
            return False
    return True


def _run(in_maps, trace=False, fast=True, diag=False, ultra=False,
         ultra_s=None, **kw):
    key = ("ultra", ultra_s) if ultra else (fast, diag)
    if key not in _cached_nc:
        _cached_nc[key] = (_build_ultra(ultra_s) if ultra
                           else _build(fast, diag))
    return run_bass_kernel_spmd(_cached_nc[key], in_maps, list(range(NCORES)),
                                trace=trace, **kw)


def _make_in_maps(u, alpha_base, beta_base, alpha_time_coeff, beta_time_coeff,
                  channel_coupling, skip_weight):
    u = np.ascontiguousarray(np.asarray(u, np.float32))
    s = 1.0 / (1.0 + np.exp(-float(np.asarray(skip_weight, np.float64))))
    M = np.asarray(channel_coupling, np.float32)
    row = np.zeros(24, np.float32)
    row[0:9] = M.reshape(-1)
    row[9:18] = (np.float32(1.0 - s) * M).reshape(-1)
    row[18] = np.float32(s)
    row[20:23] = np.float32(1.0 - s) * (np.diag(M).astype(np.float64)
                                        ** NUM_STEPS).astype(np.float32)
    scal = np.ascontiguousarray(np.tile(row[None, :], (128, 1)))
    ab = np.ascontiguousarray(np.asarray(alpha_base, np.float32))
    atc = np.ascontiguousarray(np.asarray(alpha_time_coeff, np.float32))
    bbt = np.ascontiguousarray(np.swapaxes(np.asarray(beta_base, np.float32), 1, 2))
    btct = np.ascontiguousarray(np.swapaxes(np.asarray(beta_time_coeff, np.float32), 1, 2))
    return [dict(u=np.ascontiguousarray(u[i * BL:(i + 1) * BL]), ab=ab, atc=atc,
                 bbt=bbt, btct=btct, scal=scal) for i in range(NCORES)]


def _diag_ok(channel_coupling):
    M = np.asarray(channel_coupling, np.float64)
    return bool(np.all(M == np.diag(np.diag(M))))


def kernel(u, alpha_base, beta_base, alpha_time_coeff, beta_time_coeff,
           channel_coupling, skip_weight):
    if _ultra_ok(channel_coupling, skip_weight):
        in_maps = _make_ultra_in_maps(u, skip_weight)
        res = _run(in_maps, ultra=True, ultra_s=_skip_s(skip_weight))
        return np.concatenate(
            [res.results[i]["y"].reshape(BL, C, N, N) for i in range(NCORES)],
            axis=0)
    in_maps = _make_in_maps(u, alpha_base, beta_base, alpha_time_coeff,
                            beta_time_coeff, channel_coupling, skip_weight)
    fast = _fast_ok(alpha_base, beta_base, alpha_time_coeff, beta_time_coeff)
    diag = _diag_ok(channel_coupling)
    res = _run(in_maps, fast=fast, diag=diag)
    return np.concatenate([res.results[i]["y"] for i in range(NCORES)], axis=0)

